# revision 12
# baseline (speedup 1.0000x reference)
"""Multi-head attention (B=2, S=2048, E=1024, H=16, D=64) on 8 Trainium2 cores.

Sharding: data-parallel over batch (2 groups of 4 cores), tensor-parallel
over heads within each group (4 heads per core, Megatron-style). out_proj is
sharded over its input rows; each core emits a full-width partial y and the
host sums 4 partials per batch (no device collective, no core-id logic).

The kernel is paced by the ScalarE exp stream: 128 x exp([128,1024] PSUM ->
bf16 SBUF) at ~0.7us each. Hardware findings this design is built on (all
measured on-device with reps-slope microbenchmarks):
  - The PE executes its queue IN ORDER, so any PE instruction that waits on
    a recently-issued exp stalls every later PE instruction: PV must be
    DEFERRED (here by 2 iterations) so the stream is [scores(i), PV(i-2)]
    (165us -> 110us for the bare attention loop).
  - A projection chain occupying a score-ring PSUM slot costs ~4us of ACT
    stall each (measured): the score ring must host ONLY score tiles.
    All qkv-projection chains and out_proj partials flow through the OTHER
    PSUM ring (shared with the PV accumulators) as rate-limited deferred
    work: 2 chains per iteration ahead of the first PV flush, out_proj
    chunks 1/iteration at q-chunk boundaries. Deep exp-output buffering
    (28 tiles) lets the exp stream run ahead while PV waits for the ring.
  - Concurrent row-tiled K=64 score pairs (base partitions 0/64) must keep
    each row group in ONE psum bank (par0->bank0, par1->bank1); crossing
    banks mid-stream is a hardware hazard.
  - exp with bf16 output streams ~1.5 elem/lane/cycle; (the softmax scale
    1/8 is folded into the activation; logits ~N(0,1) need no max-subtract).
  - PV lhsT = [v_h | ones]: softmax denominators accumulate free in
    partitions 64..127 of the PV accumulator.
PSUM: "st" 2x[128,1024] (scores only) + "fz" 2x[128,1024]-sized slots (PV
accumulators, projection chains, out_proj partials) = 8 banks. All PE inputs
bf16, f32 accumulation; x arrives host-pre-transposed [1024,2048] bf16.
"""

import numpy as np
from contextlib import ExitStack

import concourse.tile as tile
from concourse import bacc, mybir
from concourse.bass_utils import run_bass_kernel_spmd

B, S, E, H, D = 2, 2048, 1024, 16, 64
N_CORES = 8
HPC = 4            # heads per core
HD = HPC * D       # 256
DELTA = 2          # PV deferral depth (iterations)

F32 = mybir.dt.float32
BF16 = mybir.dt.bfloat16
EXP = mybir.ActivationFunctionType.Exp

_cached = None


class _T:
    pass


def build(reps=1, profile=False):
    nc = bacc.Bacc("TRN2", target_bir_lowering=False, debug=False,
                   num_devices=N_CORES)

    T = _T()
    T.xT_d = nc.dram_tensor("xT", [E, S], BF16, kind="ExternalInput").ap()
    wq_d = nc.dram_tensor("wq", [E, HD], BF16, kind="ExternalInput").ap()
    wk_d = nc.dram_tensor("wk", [E, HD], BF16, kind="ExternalInput").ap()
    wv_d = nc.dram_tensor("wv", [E, HD], BF16, kind="ExternalInput").ap()
    wo_d = nc.dram_tensor("wo", [HD, E], BF16, kind="ExternalInput").ap()
    T.y_d = nc.dram_tensor("y", [S, E], F32, kind="ExternalOutput").ap()

    with tile.TileContext(nc) as tc, ExitStack() as ctx:
        glob = ctx.enter_context(tc.tile_pool(name="glob", bufs=1))
        T.xT = glob.tile([128, 8, S], BF16, tag="xT")
        T.wq = glob.tile([128, 8, HD], BF16, tag="wq")
        T.wk = glob.tile([128, 8, HD], BF16, tag="wk")
        T.wv = glob.tile([128, 8, HD], BF16, tag="wv")
        T.wo = glob.tile([128, 2, E], BF16, tag="wo")
        T.qT = glob.tile([128, 2, S], BF16, tag="qT")   # q^T by head pair
        T.kT = glob.tile([128, 2, S], BF16, tag="kT")
        T.v = glob.tile([128, 16, HPC, 128], BF16, tag="v")  # [v_h | ones]
        T.xT_r = T.xT_d.rearrange("(c p) s -> p c s", p=128)

        nc.sync.dma_start(T.wk[:], wk_d.rearrange("(c p) n -> p c n", p=128))
        nc.sync.dma_start(T.wq[:], wq_d.rearrange("(c p) n -> p c n", p=128))
        nc.sync.dma_start(T.wv[:], wv_d.rearrange("(c p) n -> p c n", p=128))
        nc.sync.dma_start(T.wo[:], wo_d.rearrange("(c p) n -> p c n", p=128))
        nc.gpsimd.memset(T.v[:], 1.0)   # ones halves; v halves written/rep

        # tiny dep-free exp so the ~2.7us ACT table load runs at t~0
        warm_in = glob.tile([1, 8], F32, tag="warm_in")
        warm_out = glob.tile([1, 8], BF16, tag="warm_out")
        nc.gpsimd.memset(warm_in[:], 0.0)
        nc.scalar.activation(warm_out[:], warm_in[:], EXP, scale=1.0)

        with ExitStack() as body:
            T.mm = body.enter_context(
                tc.tile_pool(name="mm", bufs=2, space="PSUM"))
            T.fzp = body.enter_context(
                tc.tile_pool(name="fzp", bufs=2, space="PSUM"))
            T.expp = body.enter_context(tc.tile_pool(name="expp", bufs=28))
            T.recp = body.enter_context(tc.tile_pool(name="recp", bufs=4))
            T.outp = body.enter_context(tc.tile_pool(name="outp", bufs=2))
            T.ysb = body.enter_context(tc.tile_pool(name="ysb", bufs=3))

            carry = {"pend": [], "eps": [], "fz": {}, "n": 0,
                     "units": []}
            for rep in range(reps):
                _emit_body(nc, T, carry, first=(rep == 0),
                           last=(rep == reps - 1))

    nc.compile()
    return nc


def _emit_body(nc, T, carry, first, last):
    pend = carry["pend"]       # deferred PV work: (qc_key, kc, exs)
    ep_queue = carry["eps"]    # deferred out_proj chunks: (outT, row0)
    fz_by_qc = carry["fz"]

    def dma_spans():
        for z in range(4):
            nc.sync.dma_start(T.xT[:, :, z * 512:(z + 1) * 512],
                              T.xT_r[:, :, z * 512:(z + 1) * 512])

    def u_proj_T(w_t, dst, z, mc):
        # dst[:, mc, span z] = (w col-block mc)^T @ x^T span z; the chain
        # tile lives in the fz ring, NOT the score ring
        def emit():
            pp = T.fzp.tile([128, 1024], F32, tag="fz")
            for ec in range(8):
                nc.tensor.matmul(pp[:, 0:512],
                                 w_t[:, ec, mc * 128:(mc + 1) * 128],
                                 T.xT[:, ec, z * 512:(z + 1) * 512],
                                 start=(ec == 0), stop=(ec == 7))
            nc.vector.tensor_copy(dst[:, mc, z * 512:(z + 1) * 512],
                                  pp[:, 0:512])
        return emit

    def u_v(sc0):
        # v rows for seq chunks sc0..sc0+1; one merged copy
        def emit():
            pp = T.fzp.tile([128, 1024], F32, tag="fz")
            for j in range(2):
                sc = sc0 + j
                for ec in range(8):
                    nc.tensor.matmul(pp[:, j * HD:(j + 1) * HD],
                                     T.xT[:, ec, sc * 128:(sc + 1) * 128],
                                     T.wv[:, ec, :],
                                     start=(ec == 0), stop=(ec == 7))
            nc.vector.tensor_copy(
                T.v[:, sc0:sc0 + 2, :, 0:64],
                pp[:, 0:512].rearrange("p (s h d) -> p s h d", s=2, h=HPC))
        return emit

    # This rep's 24 projection chains. First rep: emitted serially up
    # front (correctness ordering for the reps=1 NEFF). Later reps: the
    # values they produce are identical to the previous rep's, so reads can
    # safely overlap the rewrite (WAR-ordered by Tile) and the chains are
    # woven uniformly through the fz ring with no deadline pressure.
    units = [u_proj_T(T.wk, T.kT, 0, 0), u_proj_T(T.wk, T.kT, 0, 1),
             u_proj_T(T.wq, T.qT, 0, 0), u_proj_T(T.wq, T.qT, 0, 1),
             u_proj_T(T.wk, T.kT, 1, 0), u_proj_T(T.wk, T.kT, 1, 1),
             u_v(0), u_v(2),
             u_proj_T(T.wk, T.kT, 2, 0), u_proj_T(T.wk, T.kT, 2, 1),
             u_v(4), u_v(6),
             u_proj_T(T.wk, T.kT, 3, 0), u_proj_T(T.wk, T.kT, 3, 1),
             u_v(8), u_v(10), u_v(12), u_v(14),
             u_proj_T(T.wq, T.qT, 1, 0), u_proj_T(T.wq, T.qT, 1, 1),
             u_proj_T(T.wq, T.qT, 2, 0), u_proj_T(T.wq, T.qT, 2, 1),
             u_proj_T(T.wq, T.qT, 3, 0), u_proj_T(T.wq, T.qT, 3, 1)]
    ui = 0

    def norm_only(fzs, qc):
        outT = T.outp.tile([128, 2, 512], BF16, tag="outT")
        rcs = []
        for hp in range(2):
            rc = T.recp.tile([64, 2, 512], F32, tag="rc")
            nc.vector.reciprocal(rc[:], fzs[hp][64:128, :, :])
            rcs.append(rc)
            for par in range(2):
                nc.vector.tensor_mul(
                    outT[par * 64:(par + 1) * 64, hp, :],
                    fzs[hp][0:64, par, :], rcs[hp][:, par, :])
        for sq in range(4):
            ep_queue.append((outT, qc * 4 + sq, sq))

    def emit_ep():
        outT, row, sq = ep_queue.pop(0)
        ep = T.fzp.tile([128, 1024], F32, tag="fz")
        epr = ep.rearrange("p (a b) -> p a b", a=2)
        for half in range(2):
            for hp in range(2):
                nc.tensor.matmul(
                    epr[:, half, :],
                    outT[:, hp, sq * 128:(sq + 1) * 128],
                    T.wo[:, hp, half * 512:(half + 1) * 512],
                    start=(hp == 0), stop=(hp == 1))
        yt = T.ysb.tile([128, E], F32, tag="y")
        nc.vector.tensor_copy(yt[:], ep[:])
        nc.sync.dma_start(T.y_d[row * 128:(row + 1) * 128, :], yt[:])

    def flush_one():
        qcn, qc2, kc2, exs2 = pend.pop(0)
        if kc2 == 0:
            fz0 = T.fzp.tile([128, 2, 512], F32, tag="fz")
            fz1 = T.fzp.tile([128, 2, 512], F32, tag="fz")
            fz_by_qc[qcn] = [fz0, fz1]
        fzs2 = fz_by_qc[qcn]
        for hp in range(2):
            for par in range(2):
                h = 2 * hp + par
                nc.tensor.matmul(
                    fzs2[hp][:, par, :],
                    T.v[:, kc2, h, :],
                    exs2[hp][:, par * 512:(par + 1) * 512],
                    start=(kc2 == 0), stop=(kc2 == 15))
        if kc2 == 15:
            norm_only(fz_by_qc.pop(qcn), qc2)

    if first:
        dma_spans()
        while ui < len(units):   # serial prologue: hard read-after-write
            units[ui]()
            ui += 1
    else:
        carry["units"].extend(units)

    uq = carry["units"]
    bu = [0]   # units emitted in the current boundary window

    for qc in range(4):
        for kc in range(16):
            if qc == 3 and kc == 0 and not last:
                dma_spans()   # next rep's x^T (WAR-ordered vs this rep)
            sts = []
            for hp in range(2):
                st = T.mm.tile([128, 1024], F32, tag="st")
                for par in range(2):   # K=64 row-packed head pair
                    lo, hi = par * 64, (par + 1) * 64
                    nc.tensor.matmul(
                        st[:, par * 512:(par + 1) * 512],
                        T.kT[lo:hi, hp, kc * 128:(kc + 1) * 128],
                        T.qT[lo:hi, hp, qc * 512:(qc + 1) * 512],
                        start=True, stop=True)
                sts.append(st)
            exs = []
            for hp in range(2):
                ex = T.expp.tile([128, 1024], BF16, tag="ex")
                nc.scalar.activation(ex[:], sts[hp][:], EXP, scale=0.125)
                exs.append(ex)
            if kc == 0:
                carry["n"] += 1
            pend.append((carry["n"], qc, kc, exs))

            # fz-ring policy: ring tiles (projection chains, out_proj
            # chunks) are only legal in the boundary window - after a
            # kc15-flush freed both fz accumulators (their releasing norm is
            # emitted) and before the next kc0-flush re-allocates them; a
            # ring tile emitted mid-accumulation would deadlock the
            # in-order PE queue on the un-emitted norm.
            window = not fz_by_qc
            ring = 2
            if window:
                while ring > 0 and ep_queue:
                    emit_ep()
                    ring -= 1
                while ring > 0 and uq and bu[0] < 6:
                    uq.pop(0)()
                    bu[0] += 1
                    ring -= 1
            nfl = 0
            while (pend and nfl < 2 and len(pend) > DELTA
                   and not (pend[0][2] == 0
                            and (ep_queue or (uq and bu[0] < 6)))):
                if pend[0][2] == 0:
                    bu[0] = 0   # closing the boundary window
                flush_one()
                nfl += 1

    if last:
        guard = 0
        while (ep_queue or pend or uq) and guard < 300:
            if ep_queue:
                emit_ep()
            elif pend:
                if pend[0][2] == 0:
                    bu[0] = 0
                flush_one()
            elif uq:
                uq.pop(0)()
            guard += 1


def _get_nc():
    global _cached
    if _cached is None:
        _cached = build()
    return _cached


def _bf16(a):
    import ml_dtypes
    return np.ascontiguousarray(a, dtype=ml_dtypes.bfloat16)


def make_in_maps(x, w_qkv, w_out):
    x = np.asarray(x, dtype=np.float32)
    w_qkv = np.asarray(w_qkv, dtype=np.float32)
    w_out = np.asarray(w_out, dtype=np.float32)
    in_maps = []
    for c in range(N_CORES):
        b, r = c // 4, c % 4
        hs = r * HD                  # first qkv column of this core's heads
        in_maps.append({
            "xT": _bf16(x[b].T),
            "wq": _bf16(w_qkv[:, hs:hs + HD]),
            "wk": _bf16(w_qkv[:, E + hs:E + hs + HD]),
            "wv": _bf16(w_qkv[:, 2 * E + hs:2 * E + hs + HD]),
            "wo": _bf16(w_out[r * HD:(r + 1) * HD, :]),
        })
    return in_maps


def assemble(results):
    y = np.zeros((B, S, E), dtype=np.float32)
    for c in range(N_CORES):
        y[c // 4] += results[c]["y"]
    return y


def kernel(x, w_qkv, w_out):
    nc = _get_nc()
    res = run_bass_kernel_spmd(nc, make_in_maps(x, w_qkv, w_out),
                               list(range(N_CORES)))
    return assemble(res.results)


# revision 13
# speedup vs baseline: 1.0334x; 1.0334x over previous
"""Multi-head attention (B=2, S=2048, E=1024, H=16, D=64) on 8 Trainium2 cores.

Sharding: data-parallel over batch (2 groups of 4 cores), tensor-parallel
over heads within each group (4 heads per core, Megatron-style). out_proj is
sharded over its input rows; each core emits a full-width partial y and the
host sums 4 partials per batch (no device collective, no core-id logic).

The kernel is paced by the ScalarE exp stream: 128 x exp([128,1024] PSUM ->
bf16 SBUF) at ~0.7us each. Hardware findings this design is built on (all
measured on-device with reps-slope microbenchmarks):
  - The PE executes its queue IN ORDER, so any PE instruction that waits on
    a recently-issued exp stalls every later PE instruction: PV must be
    DEFERRED (here by 2 iterations) so the stream is [scores(i), PV(i-2)]
    (165us -> 110us for the bare attention loop).
  - A projection chain occupying a score-ring PSUM slot costs ~4us of ACT
    stall each (measured): the score ring must host ONLY score tiles.
    All qkv-projection chains and out_proj partials flow through the OTHER
    PSUM ring (shared with the PV accumulators) as rate-limited deferred
    work: 2 chains per iteration ahead of the first PV flush, out_proj
    chunks 1/iteration at q-chunk boundaries. Deep exp-output buffering
    (28 tiles) lets the exp stream run ahead while PV waits for the ring.
  - Concurrent row-tiled K=64 score pairs (base partitions 0/64) must keep
    each row group in ONE psum bank (par0->bank0, par1->bank1); crossing
    banks mid-stream is a hardware hazard.
  - exp with bf16 output streams ~1.5 elem/lane/cycle; (the softmax scale
    1/8 is folded into the activation; logits ~N(0,1) need no max-subtract).
  - PV lhsT = [v_h | ones]: softmax denominators accumulate free in
    partitions 64..127 of the PV accumulator.
PSUM: "st" 2x[128,1024] (scores only) + "fz" 2x[128,1024]-sized slots (PV
accumulators, projection chains, out_proj partials) = 8 banks. All PE inputs
bf16, f32 accumulation; x arrives host-pre-transposed [1024,2048] bf16.
"""

import numpy as np
from contextlib import ExitStack

import concourse.tile as tile
from concourse import bacc, mybir
from concourse.bass_utils import run_bass_kernel_spmd

B, S, E, H, D = 2, 2048, 1024, 16, 64
N_CORES = 8
HPC = 4            # heads per core
HD = HPC * D       # 256
DELTA = 2          # PV deferral depth (iterations)

F32 = mybir.dt.float32
BF16 = mybir.dt.bfloat16
EXP = mybir.ActivationFunctionType.Exp

_cached = None


class _T:
    pass


def build(reps=1, profile=False):
    nc = bacc.Bacc("TRN2", target_bir_lowering=False, debug=False,
                   num_devices=N_CORES)

    T = _T()
    T.xT_d = nc.dram_tensor("xT", [128, 4, 8, 512], BF16,
                            kind="ExternalInput").ap()
    wq_d = nc.dram_tensor("wq", [E, HD], BF16, kind="ExternalInput").ap()
    wk_d = nc.dram_tensor("wk", [E, HD], BF16, kind="ExternalInput").ap()
    wv_d = nc.dram_tensor("wv", [E, HD], BF16, kind="ExternalInput").ap()
    wo_d = nc.dram_tensor("wo", [HD, E], BF16, kind="ExternalInput").ap()
    T.y_d = nc.dram_tensor("y", [S, E], F32, kind="ExternalOutput").ap()

    with tile.TileContext(nc) as tc, ExitStack() as ctx:
        glob = ctx.enter_context(tc.tile_pool(name="glob", bufs=1))
        T.xT = glob.tile([128, 4, 8, 512], BF16, tag="xT")
        T.wq = glob.tile([128, 8, HD], BF16, tag="wq")
        T.wk = glob.tile([128, 8, HD], BF16, tag="wk")
        T.wv = glob.tile([128, 8, HD], BF16, tag="wv")
        T.wo = glob.tile([128, 2, E], BF16, tag="wo")
        T.qT = glob.tile([128, 2, S], BF16, tag="qT")   # q^T by head pair
        T.kT = glob.tile([128, 2, S], BF16, tag="kT")
        T.v = glob.tile([128, 16, HPC, 128], BF16, tag="v")  # [v_h | ones]


        nc.sync.dma_start(T.wk[:], wk_d.rearrange("(c p) n -> p c n", p=128))
        nc.sync.dma_start(T.wq[:], wq_d.rearrange("(c p) n -> p c n", p=128))
        nc.sync.dma_start(T.wv[:], wv_d.rearrange("(c p) n -> p c n", p=128))
        nc.sync.dma_start(T.wo[:], wo_d.rearrange("(c p) n -> p c n", p=128))
        nc.gpsimd.memset(T.v[:], 1.0)   # ones halves; v halves written/rep

        # tiny dep-free exp so the ~2.7us ACT table load runs at t~0
        warm_in = glob.tile([1, 8], F32, tag="warm_in")
        warm_out = glob.tile([1, 8], BF16, tag="warm_out")
        nc.gpsimd.memset(warm_in[:], 0.0)
        nc.scalar.activation(warm_out[:], warm_in[:], EXP, scale=1.0)

        with ExitStack() as body:
            T.mm = body.enter_context(
                tc.tile_pool(name="mm", bufs=2, space="PSUM"))
            T.fzp = body.enter_context(
                tc.tile_pool(name="fzp", bufs=2, space="PSUM"))
            T.expp = body.enter_context(tc.tile_pool(name="expp", bufs=28))
            T.recp = body.enter_context(tc.tile_pool(name="recp", bufs=4))
            T.outp = body.enter_context(tc.tile_pool(name="outp", bufs=2))
            T.ysb = body.enter_context(tc.tile_pool(name="ysb", bufs=3))

            carry = {"pend": [], "eps": [], "fz": {}, "n": 0,
                     "units": []}
            for rep in range(reps):
                _emit_body(nc, T, carry, first=(rep == 0),
                           last=(rep == reps - 1))

    nc.compile()
    return nc


def _emit_body(nc, T, carry, first, last):
    pend = carry["pend"]       # deferred PV work: (qc_key, kc, exs)
    ep_queue = carry["eps"]    # deferred out_proj chunks: (outT, row0)
    fz_by_qc = carry["fz"]

    def dma_spans():
        # host pre-tiles x^T as [128, 4(span), 8(emb chunk), 512] so each
        # span DMA is 8KB contiguous per partition (128 descriptors, not
        # 1024x1KB - the fine-grained layout measured +60us/rep)
        for z in range(4):
            nc.sync.dma_start(T.xT[:, z], T.xT_d[:, z])

    def u_proj_T(w_t, dst, z, mc):
        # dst[:, mc, span z] = (w col-block mc)^T @ x^T span z; the chain
        # tile lives in the fz ring, NOT the score ring
        def emit():
            pp = T.fzp.tile([128, 1024], F32, tag="fz")
            for ec in range(8):
                nc.tensor.matmul(pp[:, 0:512],
                                 w_t[:, ec, mc * 128:(mc + 1) * 128],
                                 T.xT[:, z, ec, :],
                                 start=(ec == 0), stop=(ec == 7))
            nc.vector.tensor_copy(dst[:, mc, z * 512:(z + 1) * 512],
                                  pp[:, 0:512])
        return emit

    def u_v(sc0):
        # v rows for seq chunks sc0..sc0+1; one merged copy
        def emit():
            pp = T.fzp.tile([128, 1024], F32, tag="fz")
            for j in range(2):
                sc = sc0 + j
                z, so = sc // 4, (sc % 4) * 128
                for ec in range(8):
                    nc.tensor.matmul(pp[:, j * HD:(j + 1) * HD],
                                     T.xT[:, z, ec, so:so + 128],
                                     T.wv[:, ec, :],
                                     start=(ec == 0), stop=(ec == 7))
            nc.vector.tensor_copy(
                T.v[:, sc0:sc0 + 2, :, 0:64],
                pp[:, 0:512].rearrange("p (s h d) -> p s h d", s=2, h=HPC))
        return emit

    # This rep's 24 projection chains. First rep: emitted serially up
    # front (correctness ordering for the reps=1 NEFF). Later reps: the
    # values they produce are identical to the previous rep's, so reads can
    # safely overlap the rewrite (WAR-ordered by Tile) and the chains are
    # woven uniformly through the fz ring with no deadline pressure.
    units = [u_proj_T(T.wk, T.kT, 0, 0), u_proj_T(T.wk, T.kT, 0, 1),
             u_proj_T(T.wq, T.qT, 0, 0), u_proj_T(T.wq, T.qT, 0, 1),
             u_proj_T(T.wk, T.kT, 1, 0), u_proj_T(T.wk, T.kT, 1, 1),
             u_v(0), u_v(2),
             u_proj_T(T.wk, T.kT, 2, 0), u_proj_T(T.wk, T.kT, 2, 1),
             u_v(4), u_v(6),
             u_proj_T(T.wk, T.kT, 3, 0), u_proj_T(T.wk, T.kT, 3, 1),
             u_v(8), u_v(10), u_v(12), u_v(14),
             u_proj_T(T.wq, T.qT, 1, 0), u_proj_T(T.wq, T.qT, 1, 1),
             u_proj_T(T.wq, T.qT, 2, 0), u_proj_T(T.wq, T.qT, 2, 1),
             u_proj_T(T.wq, T.qT, 3, 0), u_proj_T(T.wq, T.qT, 3, 1)]
    ui = 0

    def norm_only(fzs, qc):
        outT = T.outp.tile([128, 2, 512], BF16, tag="outT")
        rcs = []
        for hp in range(2):
            rc = T.recp.tile([64, 2, 512], F32, tag="rc")
            nc.vector.reciprocal(rc[:], fzs[hp][64:128, :, :])
            rcs.append(rc)
            for par in range(2):
                nc.vector.tensor_mul(
                    outT[par * 64:(par + 1) * 64, hp, :],
                    fzs[hp][0:64, par, :], rcs[hp][:, par, :])
        for sq in range(4):
            ep_queue.append((outT, qc * 4 + sq, sq))

    def emit_ep():
        outT, row, sq = ep_queue.pop(0)
        ep = T.fzp.tile([128, 1024], F32, tag="fz")
        epr = ep.rearrange("p (a b) -> p a b", a=2)
        for half in range(2):
            for hp in range(2):
                nc.tensor.matmul(
                    epr[:, half, :],
                    outT[:, hp, sq * 128:(sq + 1) * 128],
                    T.wo[:, hp, half * 512:(half + 1) * 512],
                    start=(hp == 0), stop=(hp == 1))
        yt = T.ysb.tile([128, E], F32, tag="y")
        nc.vector.tensor_copy(yt[:], ep[:])
        nc.sync.dma_start(T.y_d[row * 128:(row + 1) * 128, :], yt[:])

    def flush_one():
        qcn, qc2, kc2, exs2 = pend.pop(0)
        if kc2 == 0:
            fz0 = T.fzp.tile([128, 2, 512], F32, tag="fz")
            fz1 = T.fzp.tile([128, 2, 512], F32, tag="fz")
            fz_by_qc[qcn] = [fz0, fz1]
        fzs2 = fz_by_qc[qcn]
        for hp in range(2):
            for par in range(2):
                h = 2 * hp + par
                nc.tensor.matmul(
                    fzs2[hp][:, par, :],
                    T.v[:, kc2, h, :],
                    exs2[hp][:, par * 512:(par + 1) * 512],
                    start=(kc2 == 0), stop=(kc2 == 15))
        if kc2 == 15:
            norm_only(fz_by_qc.pop(qcn), qc2)

    if first:
        dma_spans()
        while ui < len(units):   # serial prologue: hard read-after-write
            units[ui]()
            ui += 1
    else:
        carry["units"].extend(units)

    uq = carry["units"]
    bu = [0]   # units emitted in the current boundary window

    for qc in range(4):
        for kc in range(16):
            if qc == 3 and kc == 0 and not last:
                dma_spans()   # next rep's x^T (WAR-ordered vs this rep)
            sts = []
            for hp in range(2):
                st = T.mm.tile([128, 1024], F32, tag="st")
                for par in range(2):   # K=64 row-packed head pair
                    lo, hi = par * 64, (par + 1) * 64
                    nc.tensor.matmul(
                        st[:, par * 512:(par + 1) * 512],
                        T.kT[lo:hi, hp, kc * 128:(kc + 1) * 128],
                        T.qT[lo:hi, hp, qc * 512:(qc + 1) * 512],
                        start=True, stop=True)
                sts.append(st)
            exs = []
            for hp in range(2):
                ex = T.expp.tile([128, 1024], BF16, tag="ex")
                nc.scalar.activation(ex[:], sts[hp][:], EXP, scale=0.125)
                exs.append(ex)
            if kc == 0:
                carry["n"] += 1
            pend.append((carry["n"], qc, kc, exs))

            # fz-ring policy: ring tiles (projection chains, out_proj
            # chunks) are only legal in the boundary window - after a
            # kc15-flush freed both fz accumulators (their releasing norm is
            # emitted) and before the next kc0-flush re-allocates them; a
            # ring tile emitted mid-accumulation would deadlock the
            # in-order PE queue on the un-emitted norm.
            window = not fz_by_qc
            ring = 2
            if window:
                while ring > 0 and ep_queue:
                    emit_ep()
                    ring -= 1
                while ring > 0 and uq and bu[0] < 6:
                    uq.pop(0)()
                    bu[0] += 1
                    ring -= 1
            nfl = 0
            while (pend and nfl < 2 and len(pend) > DELTA
                   and not (pend[0][2] == 0
                            and (ep_queue or (uq and bu[0] < 6)))):
                if pend[0][2] == 0:
                    bu[0] = 0   # closing the boundary window
                flush_one()
                nfl += 1

    if last:
        guard = 0
        while (ep_queue or pend or uq) and guard < 300:
            if ep_queue:
                emit_ep()
            elif pend:
                if pend[0][2] == 0:
                    bu[0] = 0
                flush_one()
            elif uq:
                uq.pop(0)()
            guard += 1


def _get_nc():
    global _cached
    if _cached is None:
        _cached = build()
    return _cached


def _bf16(a):
    import ml_dtypes
    return np.ascontiguousarray(a, dtype=ml_dtypes.bfloat16)


def make_in_maps(x, w_qkv, w_out):
    x = np.asarray(x, dtype=np.float32)
    w_qkv = np.asarray(w_qkv, dtype=np.float32)
    w_out = np.asarray(w_out, dtype=np.float32)
    in_maps = []
    for c in range(N_CORES):
        b, r = c // 4, c % 4
        hs = r * HD                  # first qkv column of this core's heads
        xt = np.ascontiguousarray(
            x[b].T.reshape(8, 128, 4, 512).transpose(1, 2, 0, 3))
        in_maps.append({
            "xT": _bf16(xt),
            "wq": _bf16(w_qkv[:, hs:hs + HD]),
            "wk": _bf16(w_qkv[:, E + hs:E + hs + HD]),
            "wv": _bf16(w_qkv[:, 2 * E + hs:2 * E + hs + HD]),
            "wo": _bf16(w_out[r * HD:(r + 1) * HD, :]),
        })
    return in_maps


def assemble(results):
    y = np.zeros((B, S, E), dtype=np.float32)
    for c in range(N_CORES):
        y[c // 4] += results[c]["y"]
    return y


def kernel(x, w_qkv, w_out):
    nc = _get_nc()
    res = run_bass_kernel_spmd(nc, make_in_maps(x, w_qkv, w_out),
                               list(range(N_CORES)))
    return assemble(res.results)
